# revision 55
# baseline (speedup 1.0000x reference)
"""Trainium2 Bass kernel for nn_AttentionModel_47983374631276.

SDPA attention: B=2, H=16, S=2048, D=128, fp8-representable q/k/v with
per-tensor dequant scales (qs, ks, vs).

Sharding: batch*heads = 32 pairs -> 4 heads per core across 8 cores.
Each core runs its full S x S attention locally; no cross-core comm.

Per-head device algorithm:
  1. matmul1 (bf16, lossless for fp8 values): S^T[k, q] = (K^T slice).T
     @ Q^T (stationary=K^T [d,128], moving=Q^T [d,512], contraction
     d=128, PSUM f32)
  2. exp: ScalarE activation Exp over [128, 1536] PSUM chunks with the
     free affine scale = qs*ks/sqrt(D) and bias = -C_SHIFT folded in ->
     P' = exp(logits - 10) as fp16 in SBUF (fits fp16 range; 10-bit
     mantissa beats bf16; shift cancels in the softmax division).
     No row-max pass needed: |logit| <= ~15, exp stays in range.
  3. matmul2 (fp16): out_ext[q, 129] = sum_k P'^T[k,q].T @ [V | 1]
     (the ones column yields the softmax denominator for free)
  4. evac: out[q, :128] * vs / out[q, 128]  on VectorE, DMA to DRAM.

Software pipelining: phase2 of head h-1 is emitted AFTER phase1 of head
h, so the Tile scheduler keeps ScalarE (the critical engine) fed with
exp work while matmul2 instructions fill TensorE gaps.

PSUM budget (8 banks): psum1 chunks [128,1536] x2 bufs = 6 banks,
ps2 accumulators [128,129] x2 bufs = 2 banks. One accumulation group
per bank (matmul start=True clears has_written for the whole bank).
"""

import math
import os

import numpy as np
import ml_dtypes

import concourse.bacc as bacc
import concourse.bass as bass
import concourse.tile as tile
import concourse.mybir as mybir
from concourse.bass_utils import run_bass_kernel_spmd

N_CORES = 8
HEADS_PER_CORE = 4
S = 2048
D = 128
P = 128            # partitions
KT = S // P        # 16 k tiles per head
QQ = 4             # q chunks of 512 for matmul1
QW = S // QQ       # 512
SLICES = KT * QQ   # 64 matmul1 output slices of 512 per head
CHUNK_SLICES = 3   # exp chunk = 3 x 512 = 1536 elements/partition

BF16 = mybir.dt.bfloat16
FP16 = mybir.dt.float16
F32 = mybir.dt.float32

# Stash of the most recent run results / program for test harnesses.
LAST_RESULTS = None
LAST_NC = None


def _build_program(c_scale: float, vs_val: float, c_shift: float, repeat: int = 1):
    nc = bacc.Bacc()

    qT_d = nc.dram_tensor("qT", [HEADS_PER_CORE, P, S], BF16, kind="ExternalInput")
    kT_d = nc.dram_tensor("kT", [HEADS_PER_CORE, P, S], BF16, kind="ExternalInput")
    v_d = nc.dram_tensor("v", [HEADS_PER_CORE, S, D], FP16, kind="ExternalInput")
    out_d = nc.dram_tensor("out", [HEADS_PER_CORE, S, D], F32, kind="ExternalOutput")

    with tile.TileContext(nc) as tc:
        with (
            tc.tile_pool(name="io", bufs=2) as io_pool,
            tc.tile_pool(name="ptp", bufs=4) as pt_pool,
            tc.tile_pool(name="outp", bufs=4) as out_pool,
            tc.tile_pool(name="smallp", bufs=4) as small_pool,
            tc.tile_pool(name="ps1p", bufs=2, space="PSUM") as ps1_pool,
            tc.tile_pool(name="ps2p", bufs=2, space="PSUM") as ps2_pool,
        ):

            # Priority bands: the Tile list-scheduler prefers smaller
            # bass_priority among ready instructions. Keep all loads +
            # phase1 (matmul1 + exp — the ACT-critical chain) in a low
            # band so leftover phase2 matmuls never starve the next
            # head's phase1 on the in-order PE stream.
            P1_BAND = 0
            P2_BAND = 10_000_000
            HEAD_STRIDE = 100_000

            def emit_load(h, step=None):
                tc.cur_priority = P1_BAND + (h if step is None else step) * HEAD_STRIDE
                # Split the K^T/Q^T loads into column blocks so the first
                # exp chunk's matmuls depend on ~0.7us of DMA, not 2.8us
                # (Tile subtile deps track per-range coverage). v is only
                # needed by phase2, a full head later.
                kT_sb = io_pool.tile([P, S], BF16, tag="kT")
                qT_sb = io_pool.tile([P, S], BF16, tag="qT")
                if h == 0:
                    # First head: tiny leading blocks so the very first
                    # exp chunk's inputs land in ~2us instead of ~3.3us.
                    nc.sync.dma_start(kT_sb[:, : 2 * P], kT_d[h, :, : 2 * P])
                    nc.gpsimd.dma_start(qT_sb[:, :QW], qT_d[h, :, :QW])
                    nc.sync.dma_start(kT_sb[:, 2 * P : QW], kT_d[h, :, 2 * P : QW])
                    rest = [
                        (kT_sb, kT_d, nc.sync),
                        (qT_sb, qT_d, nc.gpsimd),
                    ]
                    for b in range(1, QQ):
                        sl = slice(b * QW, (b + 1) * QW)
                        for sb, dr, eng in rest:
                            eng.dma_start(sb[:, sl], dr[h, :, sl])
                else:
                    for b in range(QQ):
                        sl = slice(b * QW, (b + 1) * QW)
                        # issue on two different sequencers so the
                        # descriptors enqueue in parallel (~650ns each)
                        nc.sync.dma_start(kT_sb[:, sl], kT_d[h, :, sl])
                        nc.gpsimd.dma_start(qT_sb[:, sl], qT_d[h, :, sl])
                v_sb = io_pool.tile([P, KT, D + 1], FP16, tag="v")
                nc.sync.dma_start(
                    v_sb[:, :, :D], v_d[h].rearrange("(t p) d -> p t d", p=P)
                )
                nc.vector.memset(v_sb[:, :, D : D + 1], 1.0)
                return qT_sb, kT_sb, v_sb

            def emit_phase1(h, qT_sb, kT_sb, step=None):
                tc.cur_priority = P1_BAND + (h if step is None else step) * HEAD_STRIDE + 1000
                # P^T stored as 2 q-half tiles [P, kt_slice...] so the
                # slot WAR (bufs=4 = 2 heads in flight) couples each exp
                # chunk only to the 8 matmul2 groups reading the same
                # half two heads earlier, not to a whole phase2.
                # Each half covers q slices (qq, qq+1): 32 [128,512]
                # slices, chunked [2, 3x10]: the leading 2-slice chunk
                # restarts the ACT stream with a small PE dependency.
                halves = []
                for hh in range(2):
                    pth = pt_pool.tile([P, 2 * KT, QW], FP16, tag="pth")
                    halves.append(pth)
                    s0 = 0  # slice index within the half: s = qq_loc*KT + kt
                    for n in (2, 3, 3, 3, 3, 3, 3, 3, 3, 3, 3):
                        ps1 = ps1_pool.tile([P, CHUNK_SLICES, QW], F32, tag="ps1")
                        for j in range(n):
                            qq_loc, kt = divmod(s0 + j, KT)
                            nc.tensor.matmul(
                                ps1[:, j, :],
                                lhsT=kT_sb[:, kt * P : (kt + 1) * P],
                                rhs=qT_sb[
                                    :,
                                    (2 * hh + qq_loc) * QW : (2 * hh + qq_loc + 1) * QW,
                                ],
                                start=True,
                                stop=True,
                            )
                        nc.scalar.activation(
                            pth[:, s0 : s0 + n, :],
                            ps1[:, :n, :],
                            mybir.ActivationFunctionType.Exp,
                            scale=c_scale,
                            bias=bias_sb,
                        )
                        s0 += n
                return halves

            def emit_phase2(h, halves, v_sb, step=None, tail=False):
                tc.cur_priority = P2_BAND + (h if step is None else step) * HEAD_STRIDE
                for qt in range(KT):
                    pth = halves[qt // (2 * QQ)]
                    qq_loc, qcol = divmod(qt % (2 * QQ), QQ)
                    if tail and qt >= KT - 2:
                        # last head: phase1 is done, its psum1 slots are
                        # dead -- recycle bank 0 of each as two extra
                        # accumulators so the final groups don't
                        # serialize on the 2 ps2 slots
                        big = ps1_pool.tile([P, CHUNK_SLICES, QW], F32, tag="ps1")
                        ps2 = big[:, 0, : D + 1]
                    else:
                        ps2 = ps2_pool.tile([P, D + 1], F32, tag="ps2")
                    for kt in range(KT):
                        nc.tensor.matmul(
                            ps2,
                            lhsT=pth[:, qq_loc * KT + kt, qcol * P : (qcol + 1) * P],
                            rhs=v_sb[:, kt, :],
                            start=(kt == 0),
                            stop=(kt == KT - 1),
                        )
                    recip = small_pool.tile([P, 1], F32, tag="recip")
                    nc.vector.reciprocal(recip, ps2[:, D : D + 1])
                    o_sb = out_pool.tile([P, D], F32, tag="o")
                    nc.vector.tensor_scalar(
                        o_sb,
                        ps2[:, :D],
                        recip,
                        vs_val,
                        mybir.AluOpType.mult,
                        mybir.AluOpType.mult,
                    )
                    nc.sync.dma_start(out_d[h, qt * P : (qt + 1) * P, :], o_sb)

            bias_sb = small_pool.tile([P, 1], F32, tag="bias", bufs=1)
            nc.vector.memset(bias_sb, -c_shift)

            prev = None
            for step in range(HEADS_PER_CORE * repeat):
                h = step % HEADS_PER_CORE
                qT_sb, kT_sb, v_sb = emit_load(h, step)
                halves = emit_phase1(h, qT_sb, kT_sb, step)
                if prev is not None:
                    emit_phase2(*prev)
                prev = (h, halves, v_sb, step)
            emit_phase2(*prev, tail=True)

    nc.compile()
    return nc


def kernel(s, q, k, v, qs, ks, vs):
    global LAST_RESULTS, LAST_NC
    q = np.asarray(q, dtype=np.float32)
    k = np.asarray(k, dtype=np.float32)
    v = np.asarray(v, dtype=np.float32)
    qs = np.asarray(qs, dtype=np.float32)
    ks = np.asarray(ks, dtype=np.float32)
    vs = np.asarray(vs, dtype=np.float32)

    B, H, S_, D_ = q.shape
    assert (S_, D_) == (S, D) and B * H == N_CORES * HEADS_PER_CORE

    # fp8-representable values -> bf16 cast is lossless
    qT = np.ascontiguousarray(
        q.reshape(B * H, S, D).transpose(0, 2, 1)
    ).astype(ml_dtypes.bfloat16)
    kT = np.ascontiguousarray(
        k.reshape(B * H, S, D).transpose(0, 2, 1)
    ).astype(ml_dtypes.bfloat16)
    vb = np.ascontiguousarray(v.reshape(B * H, S, D)).astype(np.float16)

    c_scale = float(
        np.float32(qs[0]) * np.float32(ks[0]) * np.float32(1.0 / math.sqrt(D))
    )
    vs_val = float(vs[0])
    # Logit shift so P' = exp(logit - c_shift) centers each row's max near
    # 1.0 in fp16 (logit std ~ c_scale*sqrt(D) for unit-variance q/k; row
    # max of S samples ~ 3.7 sigma). The softmax division cancels the
    # shift exactly; fp16 keeps 10 mantissa bits vs bf16's 7.
    c_shift = 3.7 * math.sqrt(D) * c_scale

    nc = _build_program(c_scale, vs_val, c_shift)
    LAST_NC = nc

    in_maps = []
    for c in range(N_CORES):
        lo, hi = c * HEADS_PER_CORE, (c + 1) * HEADS_PER_CORE
        in_maps.append(
            {
                "qT": np.ascontiguousarray(qT[lo:hi]),
                "kT": np.ascontiguousarray(kT[lo:hi]),
                "v": np.ascontiguousarray(vb[lo:hi]),
            }
        )

    try:
        res = run_bass_kernel_spmd(nc, in_maps, core_ids=list(range(N_CORES)))
    except ModuleNotFoundError:
        # BASS_TRACE set but the axon NTFF hook module isn't shipped in
        # this container -- retry with tracing disabled.
        os.environ["BASS_NEVER_TRACE"] = "1"
        res = run_bass_kernel_spmd(nc, in_maps, core_ids=list(range(N_CORES)))
    LAST_RESULTS = res

    out = np.stack([r["out"] for r in res.results])  # [8, 4, S, D] f32
    return out.reshape(B, H, S, D).astype(np.float32)
